# revision 2
# baseline (speedup 1.0000x reference)
"""Multi-head attention (B=4, N=2048, DIM=1024, H=16, HD=64) on 8 TRN2 cores.

Sharding: tensor-parallel over heads - 2 heads per core. The reference omits
the output projection, so each core's output is a disjoint 128-column slice of
the final [B, N, 1024]; no collectives are needed.

Per-core schedule (v2): the ScalarE exp stream (33.5M elems/core, ~1.05us per
[128,1024] chunk) is the roofline, so everything is arranged to keep ACT fed
from the first microseconds to the last:
  - scores^T per (b, qt-wave, kt): two head-packed matmuls (tile_position row
    split) into a double-buffered [128,1024] fp32 PSUM tile; ACT exp -> bf16.
  - av (out^T = [1|v]^T @ expT, denominator in row 0) chases the exp stream
    with a 2-chunk lag instead of bunching at wave end.
  - projection is filler work scheduled by (deadline, credit): q^T/k^T via
    weight-stationary matmuls + DVE bias-add; v via weight-stationary matmuls
    (v^T) + PE transpose back to natural layout (cheaper on PE than the
    x-stationary LDWEIGHTS-bound variant).
  - tail normalize: DVE reciprocal, GpSimd partition-broadcast + bias-add
    (GpSimd is otherwise idle), DMA out.
"""

import numpy as np
import ml_dtypes

import concourse.bacc as bacc
import concourse.mybir as mybir
from concourse.bass_utils import run_bass_kernel_spmd
from concourse.tile import TileContext

B, N, DIM, H = 4, 2048, 1024, 16
HD = DIM // H
SCALE = 1.0 / np.sqrt(HD)
TOK = B * N               # 8192 tokens
NCORES = 8
HPC = H // NCORES         # heads per core = 2

BF16 = mybir.dt.bfloat16
F32 = mybir.dt.float32
AF = mybir.ActivationFunctionType

KT = 8                    # 1024 / 128 contraction tiles
NTB = 4                   # token tiles of 512 per batch
NT = B * NTB              # 16 token tiles total
QT = N // 512             # 4 q-waves per batch
KTOK_B = 16               # k-token tiles of 128 per batch
VROW = 2 * (HD + 1)       # 130: [1 | vA | 1 | vB] per 128-token tile
LAG = 2                   # av trails exp by LAG chunks

# filler PE-cost estimates (ns) for the credit scheduler
COST = {"load": 0, "qk": 1800, "vt": 1800, "vtr": 170}
CREDIT_PER_SLOT = 1050 - 222 - 426   # ACT chunk - sc slot - 2 av matmuls


def build_graph():
    nc = bacc.Bacc("TRN2", target_bir_lowering=False, debug=False)
    xt = nc.declare_dram_parameter("xt", [DIM, TOK], BF16, isOutput=False)
    wqk = nc.declare_dram_parameter("wqk", [DIM, 2 * HPC * HD], BF16, isOutput=False)
    wv = nc.declare_dram_parameter("wv", [DIM, HPC * HD], BF16, isOutput=False)
    bqk = nc.declare_dram_parameter("bqk", [2 * HPC * HD, 1], F32, isOutput=False)
    bvq = nc.declare_dram_parameter("bvq", [HD + 1, HPC], F32, isOutput=False)
    ident = nc.declare_dram_parameter("ident", [128, 128], BF16, isOutput=False)
    out = nc.declare_dram_parameter("out", [HPC, B, HD, N], F32, isOutput=True)

    with TileContext(nc) as tc:
        with (
            tc.tile_pool(name="const", bufs=1) as constp,
            tc.tile_pool(name="qk", bufs=1) as qkp,
            tc.tile_pool(name="xin", bufs=4) as xinp,
            tc.tile_pool(name="vt", bufs=2) as vtp,
            tc.tile_pool(name="exps", bufs=16) as expp,
            tc.tile_pool(name="outs", bufs=4) as outp,
            tc.tile_pool(name="rcs", bufs=4) as rcp,
            tc.tile_pool(name="bcs", bufs=4) as bcp,
        ):
            # ---- constants ----
            wqk_s = constp.tile([128, KT * 256], BF16)
            nc.sync.dma_start(
                out=wqk_s.rearrange("p (kt j) -> p kt j", kt=KT),
                in_=wqk.rearrange("(kt p) j -> p kt j", p=128))
            wv_s = constp.tile([128, KT * 128], BF16)
            nc.sync.dma_start(
                out=wv_s.rearrange("p (kt j) -> p kt j", kt=KT),
                in_=wv.rearrange("(kt p) j -> p kt j", p=128))
            bqk_s = constp.tile([128, 2], F32)
            for mt in range(2):
                nc.sync.dma_start(out=bqk_s[:, mt:mt + 1],
                                  in_=bqk[mt * 128:(mt + 1) * 128, :])
            bvq_s = constp.tile([HD + 1, HPC], F32)
            nc.sync.dma_start(out=bvq_s[:, :], in_=bvq[:, :])
            ident_s = constp.tile([128, 128], BF16)
            nc.sync.dma_start(out=ident_s[:, :], in_=ident[:, :])

            # warm the ACT exp table set off the critical path
            zz = constp.tile([1, 8], F32)
            zz2 = constp.tile([1, 8], F32)
            nc.vector.memset(zz[:, :], 0.0)
            nc.scalar.activation(zz2[:, :], zz[:, :], AF.Exp)

            q_sb = [qkp.tile([128, N], BF16, name=f"q_sb{_b}") for _b in range(B)]
            k_sb = [qkp.tile([128, N], BF16, name=f"k_sb{_b}") for _b in range(B)]
            v_sb = [qkp.tile([128, KTOK_B * VROW], BF16, name=f"v_sb{_b}")
                    for _b in range(B)]
            for _b in range(B):
                nc.gpsimd.memset(v_sb[_b][:, :], 1.0)

            with (
                tc.tile_pool(name="scps", bufs=2, space="PSUM") as scps,
                tc.tile_pool(name="avps", bufs=1, space="PSUM") as avps,
                tc.tile_pool(name="pjps", bufs=2, space="PSUM") as pjps,
            ):
                xnt_tiles = {}
                vt_tiles = {}

                def emit_group(seg):
                    kind = seg[0]
                    if kind == "load":
                        nt = seg[1]
                        xnt = xinp.tile([128, KT * 512], BF16, name="xnt")
                        nc.sync.dma_start(
                            out=xnt.rearrange("p (kt j) -> p kt j", kt=KT),
                            in_=xt.rearrange("(kt p) tok -> p kt tok", p=128)[
                                :, :, nt * 512:(nt + 1) * 512])
                        xnt_tiles[nt] = xnt
                    elif kind == "qk":
                        _, nt, mt = seg
                        bb, ntb = nt // NTB, nt % NTB
                        xnt = xnt_tiles[nt]
                        pj = pjps.tile([128, 512], F32, name="pj", tag="pj")
                        for kt in range(KT):
                            nc.tensor.matmul(
                                pj[:, :],
                                lhsT=wqk_s[:, kt * 256 + mt * 128:
                                           kt * 256 + (mt + 1) * 128],
                                rhs=xnt[:, kt * 512:(kt + 1) * 512],
                                start=(kt == 0), stop=(kt == KT - 1))
                        dst = q_sb[bb] if mt == 0 else k_sb[bb]
                        nc.vector.tensor_scalar_add(
                            dst[:, ntb * 512:(ntb + 1) * 512], pj[:, :],
                            bqk_s[:, mt:mt + 1])
                    elif kind == "vt":
                        _, nt = seg
                        xnt = xnt_tiles[nt]
                        pj = pjps.tile([128, 512], F32, name="pj", tag="pj")
                        for kt in range(KT):
                            nc.tensor.matmul(
                                pj[:, :],
                                lhsT=wv_s[:, kt * 128:(kt + 1) * 128],
                                rhs=xnt[:, kt * 512:(kt + 1) * 512],
                                start=(kt == 0), stop=(kt == KT - 1))
                        vt = vtp.tile([128, 512], BF16, name="vt")
                        nc.vector.tensor_copy(vt[:, :], pj[:, :])
                        vt_tiles[nt] = vt
                    elif kind == "vtr":
                        _, nt, j = seg
                        bb, ntb = nt // NTB, nt % NTB
                        tt = ntb * 4 + j
                        pj = pjps.tile([128, 512], F32, name="pj", tag="pj")
                        pjt = pj.bitcast(BF16)[:, 0:128]
                        nc.tensor.transpose(
                            pjt, vt_tiles[nt][:, j * 128:(j + 1) * 128],
                            ident_s[:, :])
                        nc.vector.tensor_copy(
                            v_sb[bb][:, tt * VROW + 1: tt * VROW + 1 + HD],
                            pjt[:, 0:HD])
                        nc.vector.tensor_copy(
                            v_sb[bb][:, tt * VROW + HD + 2: tt * VROW + 2 + 2 * HD],
                            pjt[:, HD:2 * HD])

                # ---- filler schedule: (deadline_slot, cost, seg) ----
                def slot(w, kt):
                    return w * 16 + kt

                filler = []
                for bb in range(B):
                    w0 = bb * QT
                    for ntb in range(NTB):
                        nt = bb * NTB + ntb
                        filler.append((slot(w0, 4 * ntb) - 8, ("load", nt)))
                        filler.append((slot(w0, 4 * ntb) - 2, ("qk", nt, 1)))
                        filler.append((slot(w0, 4 * ntb) - 2, ("vt", nt)))
                        for j in range(4):
                            filler.append((slot(w0, 4 * ntb + j) - 1,
                                           ("vtr", nt, j)))
                        filler.append((slot(w0 + ntb, 0) - 2, ("qk", nt, 0)))
                filler.sort(key=lambda x: x[0])
                from collections import deque
                filler = deque(filler)
                credit = 0.0

                def pop_filler(sidx):
                    nonlocal credit
                    while filler and filler[0][0] <= sidx:
                        _, seg = filler.popleft()
                        emit_group(seg)
                        credit -= COST[seg[0]]
                    while filler and credit >= COST[filler[0][1][0]]:
                        _, seg = filler.popleft()
                        emit_group(seg)
                        credit -= COST[seg[0]]

                def emit_av(b, pav, e, kt, h):
                    nc.tensor.matmul(
                        pav[h][:, :],
                        lhsT=v_sb[b][:, kt * VROW + h * (HD + 1):
                                     kt * VROW + (h + 1) * (HD + 1)],
                        rhs=e[:, h * 512:(h + 1) * 512],
                        start=(kt == 0), stop=(kt == KTOK_B - 1),
                        skip_group_check=True)

                def emit_tail(b, qt, pav, h):
                    dn = rcp.tile([1, 512], F32, name="dn", tag="dn")
                    nc.vector.tensor_copy(dn[0:1, :], pav[h][0:1, :])
                    rc = rcp.tile([1, 512], F32, name="rc", tag="rc")
                    nc.vector.reciprocal_approx_fast(rc[0:1, :], dn[0:1, :])
                    bcs = bcp.tile([65, 512], F32, name="bcs", tag="bcs")
                    nc.gpsimd.partition_broadcast(bcs[:, :], rc[0:1, :])
                    ot = outp.tile([65, 512], F32, name="ot", tag="ot")
                    nc.vector.tensor_mul(ot[0:65, :], pav[h][0:65, :],
                                         bcs[0:65, :])
                    ot2 = outp.tile([65, 512], F32, name="ot2", tag="ot2")
                    nc.gpsimd.tensor_scalar_add(ot2[0:65, :], ot[0:65, :],
                                                bvq_s[:, h:h + 1])
                    nc.sync.dma_start(
                        out=out[h, b, :, qt * 512:(qt + 1) * 512],
                        in_=ot2[1:65, :])

                for w in range(B * QT):
                    b, qt = w // QT, w % QT
                    qcol = qt * 512
                    pav = [avps.tile([65, 512], F32, name=f"pav{_h}",
                                     tag=f"pav{_h}", bufs=1)
                           for _h in range(2)]
                    echunks = {}
                    for kt in range(KTOK_B):
                        pop_filler(slot(w, kt))
                        s = scps.tile([128, 1024], F32, name="s", tag="s")
                        for h in range(2):
                            nc.tensor.matmul(
                                s[:, h * 512:(h + 1) * 512],
                                lhsT=k_sb[b][h * 64:(h + 1) * 64,
                                             kt * 128:(kt + 1) * 128],
                                rhs=q_sb[b][h * 64:(h + 1) * 64,
                                            qcol:qcol + 512],
                                start=True, stop=True,
                                tile_position=(h * 64, 0))
                        e = expp.tile([128, 1024], BF16, name="e", tag="e")
                        nc.scalar.activation(e[:, :], s[:, :], AF.Exp)
                        echunks[kt] = e
                        if kt >= LAG:
                            for h in range(2):
                                emit_av(b, pav, echunks[kt - LAG], kt - LAG, h)
                        credit += CREDIT_PER_SLOT
                    # flush trailing avs + tails (head A first so its tail
                    # overlaps head B's flush)
                    for h in range(2):
                        for kt in range(KTOK_B - LAG, KTOK_B):
                            emit_av(b, pav, echunks[kt], kt, h)
                        emit_tail(b, qt, pav, h)
    nc.compile()
    return nc


_GRAPH = None


def _get_graph():
    global _GRAPH
    if _GRAPH is None:
        _GRAPH = build_graph()
    return _GRAPH


def _make_in_maps(x, w_qkv, b_qkv):
    bf = ml_dtypes.bfloat16
    xt = np.ascontiguousarray(x.reshape(TOK, DIM).T).astype(bf)
    ident = np.eye(128, dtype=bf)
    in_maps = []
    for c in range(NCORES):
        hA, hB = HPC * c, HPC * c + 1
        rq = [w_qkv[h * HD:(h + 1) * HD] * SCALE for h in (hA, hB)]
        rk = [w_qkv[DIM + h * HD: DIM + (h + 1) * HD] for h in (hA, hB)]
        rv = [w_qkv[2 * DIM + h * HD: 2 * DIM + (h + 1) * HD] for h in (hA, hB)]
        wqk_c = np.ascontiguousarray(np.concatenate(rq + rk, axis=0).T).astype(bf)
        wv_c = np.ascontiguousarray(np.concatenate(rv, axis=0).T).astype(bf)
        bq = [b_qkv[h * HD:(h + 1) * HD] * SCALE for h in (hA, hB)]
        bk = [b_qkv[DIM + h * HD: DIM + (h + 1) * HD] for h in (hA, hB)]
        bvc = [b_qkv[2 * DIM + h * HD: 2 * DIM + (h + 1) * HD] for h in (hA, hB)]
        bqk_c = np.concatenate(bq + bk).astype(np.float32).reshape(-1, 1)
        bvq_c = np.zeros((HD + 1, HPC), dtype=np.float32)
        for hh in range(HPC):
            bvq_c[1:HD + 1, hh] = bvc[hh]
        in_maps.append({"xt": xt, "wqk": wqk_c, "wv": wv_c,
                        "bqk": np.ascontiguousarray(bqk_c),
                        "bvq": bvq_c, "ident": ident})
    return in_maps


def _run(x, w_qkv, b_qkv, trace=False, tmpdir=None):
    nc = _get_graph()
    in_maps = _make_in_maps(np.asarray(x, dtype=np.float32),
                            np.asarray(w_qkv, dtype=np.float32),
                            np.asarray(b_qkv, dtype=np.float32))
    res = run_bass_kernel_spmd(nc, in_maps, core_ids=list(range(NCORES)),
                               trace=trace, tmpdir=tmpdir)
    full = np.empty((B, N, DIM), dtype=np.float32)
    for c in range(NCORES):
        oc = res.results[c]["out"]          # [HPC, B, HD, N]
        full[:, :, c * HPC * HD:(c + 1) * HPC * HD] = \
            oc.transpose(1, 3, 0, 2).reshape(B, N, HPC * HD)
    return full, res


def kernel(x, w_qkv, b_qkv):
    full, _ = _run(x, w_qkv, b_qkv, trace=False)
    return full


# revision 5
# speedup vs baseline: 1.0200x; 1.0200x over previous
"""Multi-head attention (B=4, N=2048, DIM=1024, H=16, HD=64) on 8 TRN2 cores.

Sharding: tensor-parallel over heads - 2 heads per core. The reference omits
the output projection, so each core's output is a disjoint 128-column slice of
the final [B, N, 1024]; no collectives are needed.

Per-core schedule (v2): the ScalarE exp stream (33.5M elems/core, ~1.05us per
[128,1024] chunk) is the roofline, so everything is arranged to keep ACT fed
from the first microseconds to the last:
  - scores^T per (b, qt-wave, kt): two head-packed matmuls (tile_position row
    split) into a double-buffered [128,1024] fp32 PSUM tile; ACT exp -> bf16.
  - av (out^T = [1|v]^T @ expT, denominator in row 0) chases the exp stream
    with a 2-chunk lag instead of bunching at wave end.
  - projection is filler work scheduled by (deadline, credit): q^T/k^T via
    weight-stationary matmuls + DVE bias-add; v via weight-stationary matmuls
    (v^T) + PE transpose back to natural layout (cheaper on PE than the
    x-stationary LDWEIGHTS-bound variant).
  - tail normalize: DVE reciprocal, GpSimd partition-broadcast + bias-add
    (GpSimd is otherwise idle), DMA out.
"""

import numpy as np
import ml_dtypes

import concourse.bacc as bacc
import concourse.mybir as mybir
from concourse.bass_utils import run_bass_kernel_spmd
from concourse.tile import TileContext

B, N, DIM, H = 4, 2048, 1024, 16
HD = DIM // H
SCALE = 1.0 / np.sqrt(HD)
TOK = B * N               # 8192 tokens
NCORES = 8
HPC = H // NCORES         # heads per core = 2

BF16 = mybir.dt.bfloat16
F32 = mybir.dt.float32
AF = mybir.ActivationFunctionType

KT = 8                    # 1024 / 128 contraction tiles
NTB = 4                   # token tiles of 512 per batch
NT = B * NTB              # 16 token tiles total
QT = N // 512             # 4 q-waves per batch
KTOK_B = 16               # k-token tiles of 128 per batch
VROW = 2 * (HD + 1)       # 130: [1 | vA | 1 | vB] per 128-token tile
LAG = 2                   # av trails exp by LAG chunks

# filler PE-cost estimates (ns) for the credit scheduler
COST = {"load": 0, "qk": 1800, "vt": 1800, "vtr": 170}
CREDIT_PER_SLOT = 1050 - 222 - 426   # ACT chunk - sc slot - 2 av matmuls


def build_graph():
    nc = bacc.Bacc("TRN2", target_bir_lowering=False, debug=False)
    xt = nc.declare_dram_parameter("xt", [DIM, TOK], BF16, isOutput=False)
    wqk = nc.declare_dram_parameter("wqk", [DIM, 2 * HPC * HD], BF16, isOutput=False)
    wv = nc.declare_dram_parameter("wv", [DIM, HPC * HD], BF16, isOutput=False)
    bqk = nc.declare_dram_parameter("bqk", [2 * HPC * HD, 1], F32, isOutput=False)
    bvq = nc.declare_dram_parameter("bvq", [HD + 1, HPC], F32, isOutput=False)
    ident = nc.declare_dram_parameter("ident", [128, 128], BF16, isOutput=False)
    out = nc.declare_dram_parameter("out", [HPC, B, HD, N], F32, isOutput=True)

    with TileContext(nc) as tc:
        with (
            tc.tile_pool(name="const", bufs=1) as constp,
            tc.tile_pool(name="qk", bufs=1) as qkp,
            tc.tile_pool(name="xin", bufs=4) as xinp,
            tc.tile_pool(name="vt", bufs=2) as vtp,
            tc.tile_pool(name="exps", bufs=16) as expp,
            tc.tile_pool(name="outs", bufs=4) as outp,
            tc.tile_pool(name="rcs", bufs=4) as rcp,
            tc.tile_pool(name="bcs", bufs=4) as bcp,
        ):
            # ---- constants (issued from the ACT hwdge queue so they run in
            # parallel with the first x-tile loads on the sync queue; ACT is
            # idle during the ramp) ----
            wqk_s = constp.tile([128, KT * 256], BF16)
            nc.scalar.dma_start(
                out=wqk_s.rearrange("p (kt j) -> p kt j", kt=KT),
                in_=wqk.rearrange("(kt p) j -> p kt j", p=128))
            wv_s = constp.tile([128, KT * 128], BF16)
            nc.scalar.dma_start(
                out=wv_s.rearrange("p (kt j) -> p kt j", kt=KT),
                in_=wv.rearrange("(kt p) j -> p kt j", p=128))
            bqk_s = constp.tile([128, 2], F32)
            for mt in range(2):
                nc.scalar.dma_start(out=bqk_s[:, mt:mt + 1],
                                    in_=bqk[mt * 128:(mt + 1) * 128, :])
            bvq_s = constp.tile([HD + 1, HPC], F32)
            nc.scalar.dma_start(out=bvq_s[:, :], in_=bvq[:, :])
            ident_s = constp.tile([128, 128], BF16)
            nc.scalar.dma_start(out=ident_s[:, :], in_=ident[:, :])

            # warm the ACT exp table set off the critical path
            zz = constp.tile([1, 8], F32)
            zz2 = constp.tile([1, 8], F32)
            nc.vector.memset(zz[:, :], 0.0)
            nc.scalar.activation(zz2[:, :], zz[:, :], AF.Exp)

            q_sb = [qkp.tile([128, N], BF16, name=f"q_sb{_b}") for _b in range(B)]
            k_sb = [qkp.tile([128, N], BF16, name=f"k_sb{_b}") for _b in range(B)]
            v_sb = [qkp.tile([128, KTOK_B * VROW], BF16, name=f"v_sb{_b}")
                    for _b in range(B)]
            for _b in range(B):
                nc.gpsimd.memset(v_sb[_b][:, :], 1.0)

            with (
                tc.tile_pool(name="scps", bufs=2, space="PSUM") as scps,
                tc.tile_pool(name="avps", bufs=1, space="PSUM") as avps,
                tc.tile_pool(name="pjps", bufs=2, space="PSUM") as pjps,
            ):
                xnt_tiles = {}
                vt_tiles = {}

                def emit_group(seg):
                    kind = seg[0]
                    if kind == "load":
                        nt = seg[1]
                        xnt = xinp.tile([128, KT * 512], BF16, name="xnt")
                        nc.sync.dma_start(
                            out=xnt.rearrange("p (kt j) -> p kt j", kt=KT),
                            in_=xt.rearrange("(kt p) tok -> p kt tok", p=128)[
                                :, :, nt * 512:(nt + 1) * 512])
                        xnt_tiles[nt] = xnt
                    elif kind == "qk":
                        _, nt, mt = seg
                        bb, ntb = nt // NTB, nt % NTB
                        xnt = xnt_tiles[nt]
                        pj = pjps.tile([128, 512], F32, name="pj", tag="pj")
                        for kt in range(KT):
                            nc.tensor.matmul(
                                pj[:, :],
                                lhsT=wqk_s[:, kt * 256 + mt * 128:
                                           kt * 256 + (mt + 1) * 128],
                                rhs=xnt[:, kt * 512:(kt + 1) * 512],
                                start=(kt == 0), stop=(kt == KT - 1))
                        dst = q_sb[bb] if mt == 0 else k_sb[bb]
                        nc.vector.tensor_scalar_add(
                            dst[:, ntb * 512:(ntb + 1) * 512], pj[:, :],
                            bqk_s[:, mt:mt + 1])
                    elif kind == "vt":
                        _, nt = seg
                        xnt = xnt_tiles[nt]
                        pj = pjps.tile([128, 512], F32, name="pj", tag="pj")
                        for kt in range(KT):
                            nc.tensor.matmul(
                                pj[:, :],
                                lhsT=wv_s[:, kt * 128:(kt + 1) * 128],
                                rhs=xnt[:, kt * 512:(kt + 1) * 512],
                                start=(kt == 0), stop=(kt == KT - 1))
                        vt = vtp.tile([128, 512], BF16, name="vt")
                        nc.vector.tensor_copy(vt[:, :], pj[:, :])
                        vt_tiles[nt] = vt
                    elif kind == "vtr":
                        _, nt, j = seg
                        bb, ntb = nt // NTB, nt % NTB
                        tt = ntb * 4 + j
                        pj = pjps.tile([128, 512], F32, name="pj", tag="pj")
                        pjt = pj.bitcast(BF16)[:, 0:128]
                        nc.tensor.transpose(
                            pjt, vt_tiles[nt][:, j * 128:(j + 1) * 128],
                            ident_s[:, :])
                        nc.vector.tensor_copy(
                            v_sb[bb][:, tt * VROW + 1: tt * VROW + 1 + HD],
                            pjt[:, 0:HD])
                        nc.vector.tensor_copy(
                            v_sb[bb][:, tt * VROW + HD + 2: tt * VROW + 2 + 2 * HD],
                            pjt[:, HD:2 * HD])

                # ---- filler schedule: (deadline_slot, cost, seg) ----
                def slot(w, kt):
                    return w * 16 + kt

                filler = []
                for bb in range(B):
                    w0 = bb * QT
                    # batch 0 deadlines are tight (its proj IS the ramp);
                    # later batches get a one-wave-early smear so forced pops
                    # spread over the previous batch's slots instead of
                    # bunching at the batch boundary.
                    lead = 2 if bb == 0 else 18
                    for ntb in range(NTB):
                        nt = bb * NTB + ntb
                        filler.append((slot(w0, 4 * ntb) - lead - 6,
                                       ("load", nt)))
                        filler.append((slot(w0, 4 * ntb) - lead, ("qk", nt, 1)))
                        filler.append((slot(w0, 4 * ntb) - lead, ("vt", nt)))
                        for j in range(4):
                            filler.append((slot(w0, 4 * ntb + j) - lead + 1,
                                           ("vtr", nt, j)))
                        filler.append((slot(w0 + ntb, 0) - lead, ("qk", nt, 0)))
                filler.sort(key=lambda x: x[0])
                from collections import deque
                filler = deque(filler)
                credit = 0.0

                def pop_filler(sidx):
                    nonlocal credit
                    while filler and filler[0][0] <= sidx:
                        dl, seg = filler.popleft()
                        emit_group(seg)
                        if dl >= 0:
                            credit -= COST[seg[0]]
                    while filler and credit >= COST[filler[0][1][0]]:
                        _, seg = filler.popleft()
                        emit_group(seg)
                        credit -= COST[seg[0]]

                def emit_av(b, pav, e, kt, h):
                    nc.tensor.matmul(
                        pav[h][:, :],
                        lhsT=v_sb[b][:, kt * VROW + h * (HD + 1):
                                     kt * VROW + (h + 1) * (HD + 1)],
                        rhs=e[:, h * 512:(h + 1) * 512],
                        start=(kt == 0), stop=(kt == KTOK_B - 1),
                        skip_group_check=True)

                def emit_tail(b, qt, pav, h):
                    dn = rcp.tile([1, 512], F32, name="dn", tag="dn")
                    nc.vector.tensor_copy(dn[0:1, :], pav[h][0:1, :])
                    rc = rcp.tile([1, 512], F32, name="rc", tag="rc")
                    nc.vector.reciprocal_approx_fast(rc[0:1, :], dn[0:1, :])
                    bcs = bcp.tile([65, 512], F32, name="bcs", tag="bcs")
                    nc.gpsimd.partition_broadcast(bcs[:, :], rc[0:1, :])
                    ot = outp.tile([65, 512], F32, name="ot", tag="ot")
                    nc.vector.tensor_mul(ot[0:65, :], pav[h][0:65, :],
                                         bcs[0:65, :])
                    ot2 = outp.tile([65, 512], F32, name="ot2", tag="ot2")
                    nc.vector.tensor_scalar_add(ot2[0:65, :], ot[0:65, :],
                                                bvq_s[:, h:h + 1])
                    nc.sync.dma_start(
                        out=out[h, b, :, qt * 512:(qt + 1) * 512],
                        in_=ot2[1:65, :])

                for w in range(B * QT):
                    b, qt = w // QT, w % QT
                    qcol = qt * 512
                    pav = [avps.tile([65, 512], F32, name=f"pav{_h}",
                                     tag=f"pav{_h}", bufs=1)
                           for _h in range(2)]
                    echunks = {}
                    for kt in range(KTOK_B):
                        pop_filler(slot(w, kt))
                        s = scps.tile([128, 1024], F32, name="s", tag="s")
                        for h in range(2):
                            nc.tensor.matmul(
                                s[:, h * 512:(h + 1) * 512],
                                lhsT=k_sb[b][h * 64:(h + 1) * 64,
                                             kt * 128:(kt + 1) * 128],
                                rhs=q_sb[b][h * 64:(h + 1) * 64,
                                            qcol:qcol + 512],
                                start=True, stop=True,
                                tile_position=(h * 64, 0))
                        e = expp.tile([128, 1024], BF16, name="e", tag="e")
                        nc.scalar.activation(e[:, :], s[:, :], AF.Exp)
                        echunks[kt] = e
                        if kt >= LAG:
                            for h in range(2):
                                emit_av(b, pav, echunks[kt - LAG], kt - LAG, h)
                        credit += CREDIT_PER_SLOT
                    # flush trailing avs + tails (head A first so its tail
                    # overlaps head B's flush)
                    for h in range(2):
                        for kt in range(KTOK_B - LAG, KTOK_B):
                            emit_av(b, pav, echunks[kt], kt, h)
                        emit_tail(b, qt, pav, h)
    nc.compile()
    return nc


_GRAPH = None


def _get_graph():
    global _GRAPH
    if _GRAPH is None:
        _GRAPH = build_graph()
    return _GRAPH


def _make_in_maps(x, w_qkv, b_qkv):
    bf = ml_dtypes.bfloat16
    xt = np.ascontiguousarray(x.reshape(TOK, DIM).T).astype(bf)
    ident = np.eye(128, dtype=bf)
    in_maps = []
    for c in range(NCORES):
        hA, hB = HPC * c, HPC * c + 1
        rq = [w_qkv[h * HD:(h + 1) * HD] * SCALE for h in (hA, hB)]
        rk = [w_qkv[DIM + h * HD: DIM + (h + 1) * HD] for h in (hA, hB)]
        rv = [w_qkv[2 * DIM + h * HD: 2 * DIM + (h + 1) * HD] for h in (hA, hB)]
        wqk_c = np.ascontiguousarray(np.concatenate(rq + rk, axis=0).T).astype(bf)
        wv_c = np.ascontiguousarray(np.concatenate(rv, axis=0).T).astype(bf)
        bq = [b_qkv[h * HD:(h + 1) * HD] * SCALE for h in (hA, hB)]
        bk = [b_qkv[DIM + h * HD: DIM + (h + 1) * HD] for h in (hA, hB)]
        bvc = [b_qkv[2 * DIM + h * HD: 2 * DIM + (h + 1) * HD] for h in (hA, hB)]
        bqk_c = np.concatenate(bq + bk).astype(np.float32).reshape(-1, 1)
        bvq_c = np.zeros((HD + 1, HPC), dtype=np.float32)
        for hh in range(HPC):
            bvq_c[1:HD + 1, hh] = bvc[hh]
        in_maps.append({"xt": xt, "wqk": wqk_c, "wv": wv_c,
                        "bqk": np.ascontiguousarray(bqk_c),
                        "bvq": bvq_c, "ident": ident})
    return in_maps


def _run(x, w_qkv, b_qkv, trace=False, tmpdir=None):
    nc = _get_graph()
    in_maps = _make_in_maps(np.asarray(x, dtype=np.float32),
                            np.asarray(w_qkv, dtype=np.float32),
                            np.asarray(b_qkv, dtype=np.float32))
    res = run_bass_kernel_spmd(nc, in_maps, core_ids=list(range(NCORES)),
                               trace=trace, tmpdir=tmpdir)
    full = np.empty((B, N, DIM), dtype=np.float32)
    for c in range(NCORES):
        oc = res.results[c]["out"]          # [HPC, B, HD, N]
        full[:, :, c * HPC * HD:(c + 1) * HPC * HD] = \
            oc.transpose(1, 3, 0, 2).reshape(B, N, HPC * HD)
    return full, res


def kernel(x, w_qkv, b_qkv):
    full, _ = _run(x, w_qkv, b_qkv, trace=False)
    return full


# revision 7
# speedup vs baseline: 1.0307x; 1.0104x over previous
"""Multi-head attention (B=4, N=2048, DIM=1024, H=16, HD=64) on 8 TRN2 cores.

Sharding: tensor-parallel over heads - 2 heads per core. The reference omits
the output projection, so each core's output is a disjoint 128-column slice of
the final [B, N, 1024]; no collectives are needed.

Per-core schedule (v5): the ScalarE exp stream (33.5M elems/core, ~1.05us per
[128,1024] chunk) is the roofline; everything else is arranged to keep ACT fed
from the first microseconds to the last:
  - scores^T per (b, qt-wave, kt): two head-packed matmuls (tile_position row
    split) into a double-buffered [128,1024] fp32 PSUM tile; ACT exp -> bf16.
  - av (out^T = [1|v]^T @ expT, denominator in row 0) chases the exp stream
    with a 2-chunk lag instead of bunching at wave end.
  - projection runs as deadline-paced filler at single-matmul granularity (a
    whole 8-matmul group in the in-order PE queue would starve ACT for ~2us).
    Chains are emitted strictly sequentially from one global need-ordered
    list, so the 2-buffer proj PSUM ring is never clobbered mid-chain.
  - x is pre-tiled on the host so each x-tile load is one contiguous 1MB DMA.
  - tail normalize: DVE reciprocal + multiply + bias-add, GpSimd
    partition-broadcast, DMA out.
"""

import numpy as np
import ml_dtypes

import concourse.bacc as bacc
import concourse.mybir as mybir
from concourse.bass_utils import run_bass_kernel_spmd
from concourse.tile import TileContext

B, N, DIM, H = 4, 2048, 1024, 16
HD = DIM // H
SCALE = 1.0 / np.sqrt(HD)
TOK = B * N               # 8192 tokens
NCORES = 8
HPC = H // NCORES         # heads per core = 2

BF16 = mybir.dt.bfloat16
F32 = mybir.dt.float32
AF = mybir.ActivationFunctionType

KT = 8                    # 1024 / 128 contraction tiles
NTB = 4                   # token tiles of 512 per batch
NT = B * NTB              # 16 token tiles total
QT = N // 512             # 4 q-waves per batch
KTOK_B = 16               # k-token tiles of 128 per batch
VROW = 2 * (HD + 1)       # 130: [1 | vA | 1 | vB] per 128-token tile
LAG = 2                   # av trails exp by LAG chunks

COST = {"load": 0, "qkmm": 230, "vmm": 160}
CREDIT_PER_SLOT = 1050 - 222 - 426   # ACT chunk - sc slot - 2 av matmuls


def build_graph():
    nc = bacc.Bacc("TRN2", target_bir_lowering=False, debug=False)
    xtt = nc.declare_dram_parameter("xtt", [NT, 128, KT * 512], BF16,
                                    isOutput=False)
    wqk = nc.declare_dram_parameter("wqk", [DIM, 2 * HPC * HD], BF16, isOutput=False)
    wv = nc.declare_dram_parameter("wv", [DIM, HPC * HD], BF16, isOutput=False)
    bqk = nc.declare_dram_parameter("bqk", [2 * HPC * HD, 1], F32, isOutput=False)
    bvq = nc.declare_dram_parameter("bvq", [HD + 1, HPC], F32, isOutput=False)
    out = nc.declare_dram_parameter("out", [HPC, B, HD, N], F32, isOutput=True)

    with TileContext(nc) as tc:
        with (
            tc.tile_pool(name="const", bufs=1) as constp,
            tc.tile_pool(name="qk", bufs=1) as qkp,
            tc.tile_pool(name="xin", bufs=4) as xinp,
            tc.tile_pool(name="exps", bufs=16) as expp,
            tc.tile_pool(name="outs", bufs=4) as outp,
            tc.tile_pool(name="rcs", bufs=4) as rcp,
            tc.tile_pool(name="bcs", bufs=4) as bcp,
        ):
            # ---- constants (ACT hwdge queue: parallel with x loads on sync,
            # ACT is idle during the ramp) ----
            wqk_s = constp.tile([128, KT * 256], BF16)
            nc.scalar.dma_start(
                out=wqk_s.rearrange("p (kt j) -> p kt j", kt=KT),
                in_=wqk.rearrange("(kt p) j -> p kt j", p=128))
            wv_s = constp.tile([128, KT * 128], BF16)
            nc.scalar.dma_start(
                out=wv_s.rearrange("p (kt j) -> p kt j", kt=KT),
                in_=wv.rearrange("(kt p) j -> p kt j", p=128))
            bqk_s = constp.tile([128, 2], F32)
            for mt in range(2):
                nc.scalar.dma_start(out=bqk_s[:, mt:mt + 1],
                                    in_=bqk[mt * 128:(mt + 1) * 128, :])
            bvq_s = constp.tile([HD + 1, HPC], F32)
            nc.scalar.dma_start(out=bvq_s[:, :], in_=bvq[:, :])

            # warm the ACT exp table set off the critical path
            zz = constp.tile([1, 8], F32)
            zz2 = constp.tile([1, 8], F32)
            nc.vector.memset(zz[:, :], 0.0)
            nc.scalar.activation(zz2[:, :], zz[:, :], AF.Exp)

            q_sb = [qkp.tile([128, N], BF16, name=f"q_sb{_b}") for _b in range(B)]
            k_sb = [qkp.tile([128, N], BF16, name=f"k_sb{_b}") for _b in range(B)]
            v_sb = [qkp.tile([128, KTOK_B * VROW], BF16, name=f"v_sb{_b}")
                    for _b in range(B)]
            for _b in range(B):
                nc.gpsimd.memset(v_sb[_b][:, :], 1.0)

            with (
                tc.tile_pool(name="scps", bufs=2, space="PSUM") as scps,
                tc.tile_pool(name="avps", bufs=1, space="PSUM") as avps,
                tc.tile_pool(name="pjps", bufs=2, space="PSUM") as pjps,
            ):
                xnt_tiles = {}
                pj_open = {}

                def emit_item(seg):
                    kind = seg[0]
                    if kind == "load":
                        nt = seg[1]
                        xnt = xinp.tile([128, KT * 512], BF16, name="xnt")
                        nc.sync.dma_start(out=xnt[:, :], in_=xtt[nt])
                        xnt_tiles[nt] = xnt
                    elif kind == "qkmm":
                        _, nt, mt, kt = seg
                        bb, ntb = nt // NTB, nt % NTB
                        if kt == 0:
                            assert not pj_open, f"open chain at {seg}: {pj_open}"
                            pj_open[(nt, mt)] = pjps.tile(
                                [128, 512], F32, name="pj", tag="pj")
                        pj = pj_open[(nt, mt)]
                        nc.tensor.matmul(
                            pj[:, :],
                            lhsT=wqk_s[:, kt * 256 + mt * 128:
                                       kt * 256 + (mt + 1) * 128],
                            rhs=xnt_tiles[nt][:, kt * 512:(kt + 1) * 512],
                            start=(kt == 0), stop=(kt == KT - 1),
                            skip_group_check=True)
                        if kt == KT - 1:
                            dst = q_sb[bb] if mt == 0 else k_sb[bb]
                            nc.vector.tensor_scalar_add(
                                dst[:, ntb * 512:(ntb + 1) * 512], pj[:, :],
                                bqk_s[:, mt:mt + 1])
                            del pj_open[(nt, mt)]
                    elif kind == "vmm":
                        _, nt, sub, kt = seg
                        bb, ntb = nt // NTB, nt % NTB
                        if kt == 0:
                            assert not pj_open, f"open chain at {seg}: {pj_open}"
                            pj_open[(nt, 2, sub)] = pjps.tile(
                                [128, 512], F32, name="pj", tag="pj")
                        pj = pj_open[(nt, 2, sub)]
                        nc.tensor.matmul(
                            pj[:, 0:128],
                            lhsT=xnt_tiles[nt][:, kt * 512 + sub * 128:
                                               kt * 512 + (sub + 1) * 128],
                            rhs=wv_s[:, kt * 128:(kt + 1) * 128],
                            start=(kt == 0), stop=(kt == KT - 1),
                            skip_group_check=True)
                        if kt == KT - 1:
                            tt = ntb * 4 + sub
                            nc.vector.tensor_copy(
                                v_sb[bb][:, tt * VROW + 1: tt * VROW + 1 + HD],
                                pj[:, 0:HD])
                            nc.vector.tensor_copy(
                                v_sb[bb][:, tt * VROW + HD + 2:
                                         tt * VROW + 2 + 2 * HD],
                                pj[:, HD:2 * HD])
                            del pj_open[(nt, 2, sub)]

                # ---- filler: one global need-ordered list of atomic chains,
                # emitted strictly in order at single-matmul grain ----
                def slot(w, kt):
                    return w * 16 + kt

                chains = []   # (need_slot, seq_tiebreak, [items...])
                seqno = 0
                for bb in range(B):
                    w0 = bb * QT
                    sh = 0 if bb == 0 else 16
                    for ntb in range(NTB):
                        nt = bb * NTB + ntb
                        s0 = slot(w0, 4 * ntb)
                        chains.append((s0 - 6 - sh, seqno, [("load", nt)]))
                        seqno += 1
                        if ntb == 0:
                            chains.append((s0 - sh, seqno,
                                           [("qkmm", nt, 0, kt)
                                            for kt in range(KT)]))
                            seqno += 1
                        chains.append((s0 - sh, seqno,
                                       [("qkmm", nt, 1, kt)
                                        for kt in range(KT)]))
                        seqno += 1
                        for sub in range(4):
                            chains.append((slot(w0, 4 * ntb + sub) + 1 - sh,
                                           seqno,
                                           [("vmm", nt, sub, kt)
                                            for kt in range(KT)]))
                            seqno += 1
                        if ntb >= 1:
                            chains.append((slot(w0 + ntb, 0) - sh, seqno,
                                           [("qkmm", nt, 0, kt)
                                            for kt in range(KT)]))
                            seqno += 1
                chains.sort(key=lambda c: (c[0], c[1]))
                filler = []
                prev_dl = -10**9
                for need, _, items in chains:
                    n = len(items)
                    for i, it in enumerate(items):
                        dl = need - (n - 1 - i) // 2 - 1
                        dl = max(dl, prev_dl)
                        prev_dl = dl
                        filler.append((dl, it))
                from collections import deque
                filler = deque(filler)
                credit = 0.0

                def pop_filler(sidx):
                    nonlocal credit
                    while filler and filler[0][0] <= sidx:
                        dl, seg = filler.popleft()
                        emit_item(seg)
                        if dl >= 0:
                            credit -= COST[seg[0]]
                    while filler and credit >= COST[filler[0][1][0]]:
                        _, seg = filler.popleft()
                        emit_item(seg)
                        credit -= COST[seg[0]]

                def emit_av(b, pav, e, kt, h):
                    nc.tensor.matmul(
                        pav[h][:, :],
                        lhsT=v_sb[b][:, kt * VROW + h * (HD + 1):
                                     kt * VROW + (h + 1) * (HD + 1)],
                        rhs=e[:, h * 512:(h + 1) * 512],
                        start=(kt == 0), stop=(kt == KTOK_B - 1),
                        skip_group_check=True)

                def emit_tail(b, qt, pav, h):
                    dn = rcp.tile([1, 512], F32, name="dn", tag="dn")
                    nc.vector.tensor_copy(dn[0:1, :], pav[h][0:1, :])
                    rc = rcp.tile([1, 512], F32, name="rc", tag="rc")
                    nc.vector.reciprocal_approx_fast(rc[0:1, :], dn[0:1, :])
                    bcs = bcp.tile([65, 512], F32, name="bcs", tag="bcs")
                    nc.gpsimd.partition_broadcast(bcs[:, :], rc[0:1, :])
                    ot = outp.tile([65, 512], F32, name="ot", tag="ot")
                    nc.vector.tensor_mul(ot[0:65, :], pav[h][0:65, :],
                                         bcs[0:65, :])
                    ot2 = outp.tile([65, 512], F32, name="ot2", tag="ot2")
                    nc.vector.tensor_scalar_add(ot2[0:65, :], ot[0:65, :],
                                                bvq_s[:, h:h + 1])
                    nc.sync.dma_start(
                        out=out[h, b, :, qt * 512:(qt + 1) * 512],
                        in_=ot2[1:65, :])

                for w in range(B * QT):
                    b, qt = w // QT, w % QT
                    qcol = qt * 512
                    pav = [avps.tile([65, 512], F32, name=f"pav{_h}",
                                     tag=f"pav{_h}", bufs=1)
                           for _h in range(2)]
                    echunks = {}
                    for kt in range(KTOK_B):
                        pop_filler(slot(w, kt))
                        s = scps.tile([128, 1024], F32, name="s", tag="s")
                        for h in range(2):
                            nc.tensor.matmul(
                                s[:, h * 512:(h + 1) * 512],
                                lhsT=k_sb[b][h * 64:(h + 1) * 64,
                                             kt * 128:(kt + 1) * 128],
                                rhs=q_sb[b][h * 64:(h + 1) * 64,
                                            qcol:qcol + 512],
                                start=True, stop=True,
                                tile_position=(h * 64, 0),
                                skip_group_check=True)
                        e = expp.tile([128, 1024], BF16, name="e", tag="e")
                        nc.scalar.activation(e[:, :], s[:, :], AF.Exp)
                        echunks[kt] = e
                        if kt >= LAG:
                            for h in range(2):
                                emit_av(b, pav, echunks[kt - LAG], kt - LAG, h)
                        credit += CREDIT_PER_SLOT
                    for h in range(2):
                        for kt in range(KTOK_B - LAG, KTOK_B):
                            emit_av(b, pav, echunks[kt], kt, h)
                        emit_tail(b, qt, pav, h)
    nc.compile()
    return nc


_GRAPH = None


def _get_graph():
    global _GRAPH
    if _GRAPH is None:
        _GRAPH = build_graph()
    return _GRAPH


def _make_in_maps(x, w_qkv, b_qkv):
    bf = ml_dtypes.bfloat16
    xT = np.ascontiguousarray(x.reshape(TOK, DIM).T).astype(bf)  # [DIM, TOK]
    # xtt[nt, p, kt*512 + j] = xT[kt*128 + p, nt*512 + j]
    xtt = np.ascontiguousarray(
        xT.reshape(KT, 128, NT, 512).transpose(2, 1, 0, 3).reshape(
            NT, 128, KT * 512))
    in_maps = []
    for c in range(NCORES):
        hA, hB = HPC * c, HPC * c + 1
        rq = [w_qkv[h * HD:(h + 1) * HD] * SCALE for h in (hA, hB)]
        rk = [w_qkv[DIM + h * HD: DIM + (h + 1) * HD] for h in (hA, hB)]
        rv = [w_qkv[2 * DIM + h * HD: 2 * DIM + (h + 1) * HD] for h in (hA, hB)]
        wqk_c = np.ascontiguousarray(np.concatenate(rq + rk, axis=0).T).astype(bf)
        wv_c = np.ascontiguousarray(np.concatenate(rv, axis=0).T).astype(bf)
        bq = [b_qkv[h * HD:(h + 1) * HD] * SCALE for h in (hA, hB)]
        bk = [b_qkv[DIM + h * HD: DIM + (h + 1) * HD] for h in (hA, hB)]
        bvc = [b_qkv[2 * DIM + h * HD: 2 * DIM + (h + 1) * HD] for h in (hA, hB)]
        bqk_c = np.concatenate(bq + bk).astype(np.float32).reshape(-1, 1)
        bvq_c = np.zeros((HD + 1, HPC), dtype=np.float32)
        for hh in range(HPC):
            bvq_c[1:HD + 1, hh] = bvc[hh]
        in_maps.append({"xtt": xtt, "wqk": wqk_c, "wv": wv_c,
                        "bqk": np.ascontiguousarray(bqk_c),
                        "bvq": bvq_c})
    return in_maps


def _run(x, w_qkv, b_qkv, trace=False, tmpdir=None):
    nc = _get_graph()
    in_maps = _make_in_maps(np.asarray(x, dtype=np.float32),
                            np.asarray(w_qkv, dtype=np.float32),
                            np.asarray(b_qkv, dtype=np.float32))
    res = run_bass_kernel_spmd(nc, in_maps, core_ids=list(range(NCORES)),
                               trace=trace, tmpdir=tmpdir)
    full = np.empty((B, N, DIM), dtype=np.float32)
    for c in range(NCORES):
        oc = res.results[c]["out"]          # [HPC, B, HD, N]
        full[:, :, c * HPC * HD:(c + 1) * HPC * HD] = \
            oc.transpose(1, 3, 0, 2).reshape(B, N, HPC * HD)
    return full, res


def kernel(x, w_qkv, b_qkv):
    full, _ = _run(x, w_qkv, b_qkv, trace=False)
    return full
